# revision 42
# baseline (speedup 1.0000x reference)
"""Trainium2 Bass kernel for DiagonalGMMPosterior (vq_codebook).

Reference computation (per batch b, descriptor n, cluster k):
    dist[k,n]  = sum_d (x[d,n] - mu_n[k,d])^2 * exp(-log_sigma[k,d])
    logits     = -dist + log_alpha[k] - 0.5 * sum_d log_sigma[k,d]
    out[k,n]   = softmax_k(logits)

Device strategy (8 NeuronCores, data-parallel over the batch axis):
  * Host folds all (K,D) parameter math into two GEMM weight matrices and a
    per-cluster constant, then CENTERS them across K.  Softmax is invariant
    to per-n shifts, so subtracting the K-mean of the logits (a rank-1
    update folded into the weights on host) bounds the logits to ~±16 and
    removes the need for a per-n max reduction entirely — no transposes,
    no partition-axis max.
  * Per core: for each 1024-column tile of x (D=128 on partitions, two
    512-wide PSUM banks side by side; float32r streams fp32 through the
    PE at 1 cycle/row instead of 4):
       psum = W1^T @ x^2 + W2^T @ x          (TensorE, PSUM accumulate)
       e    = exp(psum + cc)                 (ScalarE, per-partition bias)
       s    = ones_K^T @ e                   (TensorE: partition-sum)
       r    = 1/s                            (VectorE)
       rb   = ones_1K^T @ r                  (TensorE: partition-broadcast)
       out  = e * rb                         (VectorE)
"""

import numpy as np

import concourse.bacc as bacc
import concourse.bass as bass
import concourse.tile as tile
from concourse import mybir
from concourse.bass_utils import run_bass_kernel_spmd

B, D, N, K = 16, 128, 16384, 64
NCORES = 8
BPC = B // NCORES  # batches per core
NT = 512           # one PSUM bank of fp32
PAIR = 2 * NT      # two banks processed per iteration

F32 = mybir.dt.float32
F32R = mybir.dt.float32r
BF16 = mybir.dt.bfloat16

_CACHE = {}


def _build_nc():
    # Bacc (not raw Bass): its compile() pass legalizes Tile's multi-wait
    # instructions (move_matmul_waits_to_ldweights + generate_event_semaphores)
    # down to the 1-wait-per-instruction hardware limit.
    nc = bacc.Bacc("TRN2", target_bir_lowering=False, debug=False)
    x_in = nc.declare_dram_parameter("x", [BPC, D, N], F32R, isOutput=False)
    w1_in = nc.declare_dram_parameter("w1", [D, K], F32R, isOutput=False)
    w2_in = nc.declare_dram_parameter("w2", [D, K], F32R, isOutput=False)
    cc_in = nc.declare_dram_parameter("cc", [K, 1], F32, isOutput=False)
    ones_in = nc.declare_dram_parameter("ones_kk", [K, K], F32R, isOutput=False)
    # bf16 output halves the store traffic; the host widens back to fp32
    out_ext = nc.declare_dram_parameter("out", [BPC, K, N], BF16, isOutput=True)

    with tile.TileContext(nc) as tc:
        with (
            tc.tile_pool(name="consts", bufs=1) as consts,
            tc.tile_pool(name="xp", bufs=8) as xp,
            tc.tile_pool(name="ep", bufs=6) as ep,
            tc.tile_pool(name="op", bufs=6) as op,
            tc.tile_pool(name="rp", bufs=6) as rp,
            tc.tile_pool(name="pd", bufs=2, space="PSUM") as pdp,
            tc.tile_pool(name="pb", bufs=2, space="PSUM") as pbp,
        ):
            w1_sb = consts.tile([D, K], F32R)
            nc.sync.dma_start(out=w1_sb, in_=w1_in[:, :])
            w2_sb = consts.tile([D, K], F32R)
            nc.sync.dma_start(out=w2_sb, in_=w2_in[:, :])
            cc_sb = consts.tile([K, 1], F32)
            nc.sync.dma_start(out=cc_sb, in_=cc_in[:, :])
            ones_kk = consts.tile([K, K], F32R)
            nc.sync.dma_start(out=ones_kk, in_=ones_in[:, :])

            n_pairs = N // PAIR  # 16 per batch row
            pairs = [(b, p) for b in range(BPC) for p in range(n_pairs)]
            NP = len(pairs)
            st = [dict() for _ in range(NP)]

            # software-pipelined emission: each engine's in-order stream
            # interleaves stages of consecutive pairs so no stage
            # head-of-line-blocks the next pair's earlier stage
            def s0_load(i):
                b, p = pairs[i]
                n0 = p * PAIR
                xt = xp.tile([D, PAIR], F32R, tag="xt")
                nc.sync.dma_start(out=xt, in_=x_in[b, :, n0 : n0 + PAIR])
                st[i]["xt"] = xt

            def s1_square(i):
                xt = st[i]["xt"]
                xsq = xp.tile([D, PAIR], F32R, tag="xsq")
                nc.scalar.activation(
                    out=xsq, in_=xt.bitcast(F32),
                    func=mybir.ActivationFunctionType.Square,
                )
                st[i]["xsq"] = xsq

            def s2_dist(i):
                xt, xsq = st[i]["xt"], st[i]["xsq"]
                # dist-difference GEMM: two 512-wide halves, one PSUM
                # bank each, both at base partition 0 (f32r matmuls
                # reject other output base partitions)
                pd_t = pdp.tile([K, PAIR], F32, tag="pd")
                for h in range(2):
                    sl = slice(h * NT, (h + 1) * NT)
                    nc.tensor.matmul(
                        pd_t[:, sl], w1_sb[:, :], xsq[:, sl],
                        start=True, stop=False,
                    )
                    nc.tensor.matmul(
                        pd_t[:, sl], w2_sb[:, :], xt[:, sl],
                        start=False, stop=True,
                    )
                st[i]["pd"] = pd_t

            def s3_exp(i):
                pd_t = st[i].pop("pd")
                et = ep.tile([K, PAIR], F32R, tag="et")
                nc.scalar.activation(
                    out=et, in_=pd_t,
                    func=mybir.ActivationFunctionType.Exp,
                    bias=cc_sb, scale=1.0,
                )
                st[i]["et"] = et
                st[i].pop("xt")
                st[i].pop("xsq")

            def s4_den(i):
                et = st[i]["et"]
                # denominator, summed over k AND broadcast to all 64
                # partitions in one shot: ones_kk^T @ et
                pb_t = pbp.tile([K, PAIR], F32, tag="pb")
                for h in range(2):
                    sl = slice(h * NT, (h + 1) * NT)
                    nc.tensor.matmul(
                        pb_t[:, sl], ones_kk[:, :], et[:, sl],
                        start=True, stop=True,
                    )
                st[i]["pb"] = pb_t

            def s5_recip(i):
                pb_t = st[i].pop("pb")
                r_all = rp.tile([K, PAIR], F32, tag="r")
                # ~18-bit-accurate custom-DVE reciprocal, ~5x faster than
                # the exact iterative-divide reciprocal(); the sum is
                # always >= 1 (mean-centered logits), so the undefined
                # edge cases (0/denorm/inf) cannot occur
                nc.vector.reciprocal_approx_fast(out=r_all, in_=pb_t)
                st[i]["r"] = r_all

            def s6_mult(i):
                et, r_all = st[i].pop("et"), st[i].pop("r")
                ot = op.tile([K, PAIR], BF16, tag="ot")
                nc.vector.tensor_mul(ot, et.bitcast(F32), r_all)
                st[i]["ot"] = ot

            def s7_store(i):
                b, p = pairs[i]
                n0 = p * PAIR
                ot = st[i].pop("ot")
                nc.sync.dma_start(
                    out=out_ext[b, :, n0 : n0 + PAIR], in_=ot[:, :]
                )

            stages = [
                s0_load, s1_square, s2_dist, s3_exp,
                s4_den, s5_recip, s6_mult, s7_store,
            ]
            NS = len(stages)
            # downstream stages emitted first within each tick so no
            # engine's in-order queue blocks a later pair's earlier stage
            for tick in range(NP + NS - 1):
                for k in reversed(range(NS)):
                    i = tick - k
                    if 0 <= i < NP:
                        stages[k](i)
    nc.compile()
    return nc


def _host_params(mu, log_sigma, log_alpha):
    mu64 = mu.astype(np.float64)
    mu_n = mu64 / np.maximum(
        np.linalg.norm(mu64, axis=1, keepdims=True), 1e-12
    )
    sinv = np.exp(-log_sigma.astype(np.float64))  # (K, D)
    a1 = -sinv                                    # coeff of x^2 in logits
    a2 = 2.0 * mu_n * sinv                        # coeff of x
    c = (
        -np.sum(mu_n * mu_n * sinv, axis=1)
        + log_alpha.astype(np.float64)
        - 0.5 * np.sum(log_sigma.astype(np.float64), axis=1)
    )
    # center across K: softmax is invariant to per-n shifts, and this keeps
    # the on-device logits within exp()'s comfortable fp32 range (~±16)
    a1c = a1 - a1.mean(axis=0, keepdims=True)
    a2c = a2 - a2.mean(axis=0, keepdims=True)
    ccv = c - c.mean()
    w1 = np.ascontiguousarray(a1c.T, dtype=np.float32)  # (D, K)
    w2 = np.ascontiguousarray(a2c.T, dtype=np.float32)  # (D, K)
    cc = ccv.astype(np.float32).reshape(K, 1)
    return w1, w2, cc


def _in_maps(x, mu, log_sigma, log_alpha):
    x = np.ascontiguousarray(np.asarray(x), dtype=np.float32)
    w1, w2, cc = _host_params(
        np.asarray(mu), np.asarray(log_sigma), np.asarray(log_alpha)
    )
    ones_kk = np.ones((K, K), dtype=np.float32)
    return [
        {
            "x": x[i * BPC : (i + 1) * BPC],
            "w1": w1,
            "w2": w2,
            "cc": cc,
            "ones_kk": ones_kk,
        }
        for i in range(NCORES)
    ]


def kernel(x, mu, log_sigma, log_alpha):
    if "nc" not in _CACHE:
        _CACHE["nc"] = _build_nc()
    nc = _CACHE["nc"]
    in_maps = _in_maps(x, mu, log_sigma, log_alpha)
    res = run_bass_kernel_spmd(nc, in_maps, list(range(NCORES))).results
    out = np.concatenate(
        [np.asarray(res[i]["out"]) for i in range(NCORES)], axis=0
    )
    return out.astype(np.float32)
